# revision 16
# baseline (speedup 1.0000x reference)
"""GCN (3-layer, BN + gated skip + mean-pool + MLP head) on 8 TRN2 NeuronCores.

Strategy: shard nodes+edges 8-way by dst node. Per layer, AllGather the bf16
node-feature table into every core's HBM, gather edge source rows with
indirect DMA, segmented-sum via one-hot matmuls in PSUM (GCN normalization
coefficients folded into the one-hot; self loops are extra edges), BatchNorm
stats via a tiny AllReduce, fused scale/bias/ReLU on the scalar engine, gated
skip via matmuls, graph mean-pool via one-hot matmuls, replicated MLP head.

Activations are kept feature-on-partition ([128, nodes]) so all per-feature
affine work is per-partition scalars.
"""
import os
import sys

for _p in ("/opt/trn_rl_repo",):
    if _p not in sys.path and os.path.isdir(_p):
        sys.path.insert(0, _p)

import numpy as np
import ml_dtypes

BF16 = ml_dtypes.bfloat16

import concourse.bass as bass
import concourse.bacc as bacc
import concourse.tile as tile
import concourse.mybir as mybir
from concourse import bass_utils
from concourse.masks import make_identity

F32 = mybir.dt.float32
BF = mybir.dt.bfloat16
I32 = mybir.dt.int32
I16 = mybir.dt.int16

EPS = 1e-5


class Cfg:
    def __init__(self, n_nodes=100000, n_edges=1600000, n_graphs=1000,
                 in_dim=64, hid=128, ex_dim=32, out_dim=10, n_cores=8,
                 pool_w=256, gpad=1024, gcall=4096):
        self.N, self.E, self.G = n_nodes, n_edges, n_graphs
        self.IN, self.HID, self.EX, self.OUT = in_dim, hid, ex_dim, out_dim
        self.C = n_cores
        assert self.N % self.C == 0
        self.NPC = self.N // self.C                 # owned nodes per core
        self.T = (self.NPC + 127) // 128            # dst tiles per core
        self.SPAD = self.T * 128                    # padded nodes per core
        self.W = pool_w                             # pooling window (graph span per core)
        self.GPAD = gpad                            # padded graph count
        self.GX = gpad + pool_w                     # pooled_glob columns
        self.GCALL = gcall                          # indices per gather call
        assert self.GPAD >= self.G


FULL = Cfg()


# ---------------------------------------------------------------- host prep
def preprocess(cfg, x, eFeature, edge_index, batch, weights):
    """Shard/reorder inputs. Returns (in_maps, params)."""
    C, N, NPC, SPAD, T = cfg.C, cfg.N, cfg.NPC, cfg.SPAD, cfg.T
    src = np.asarray(edge_index[0], dtype=np.int64)
    dst = np.asarray(edge_index[1], dtype=np.int64)
    batch = np.asarray(batch, dtype=np.int64)
    x = np.asarray(x, dtype=np.float32)

    deg = (np.bincount(dst, minlength=N) + 1.0).astype(np.float64)
    dinv = (1.0 / np.sqrt(deg)).astype(np.float32)

    nodes = np.arange(N, dtype=np.int64)
    SRC = np.concatenate([src, nodes])
    DST = np.concatenate([dst, nodes])
    CO = np.concatenate([(dinv[src] * dinv[dst]).astype(np.float32),
                         (dinv * dinv).astype(np.float32)])

    owner = DST // NPC
    slot = DST - owner * NPC
    tl = slot >> 7
    dloc = slot & 127
    gidx = (SRC // NPC) * SPAD + (SRC % NPC)        # padded table row id

    NB = -(-(C * SPAD) // 32768)                    # int16-addressable buckets
    assert (C * SPAD) % NB == 0
    BROWS = (C * SPAD) // NB
    GCH = cfg.GCALL // 128                          # chunks per gather call

    bkt = gidx // BROWS
    lidx = (gidx - bkt * BROWS).astype(np.int16)

    key = (owner * NB + bkt) * T + tl               # (core, bucket, tile)
    counts3 = np.bincount(key, minlength=C * NB * T).reshape(C, NB, T)
    K_bt = -(-counts3.max(axis=0) // 128)           # [NB, T]
    chunks_b = K_bt.sum(axis=1)
    K_bt[:, T - 1] += (-chunks_b) % GCH             # pad streams to call size
    chunks_b = K_bt.sum(axis=1)
    NCHUNK = int(chunks_b.sum())
    NCALLS = NCHUNK // GCH
    base_bt = np.concatenate([[0], np.cumsum(K_bt.ravel())[:-1]]).reshape(NB, T)
    call_bucket = np.repeat(np.arange(NB), chunks_b // GCH)

    order = np.argsort(key, kind="stable")
    ks = key[order]
    run_start = np.r_[0, np.flatnonzero(np.diff(ks)) + 1]
    run_len = np.diff(np.r_[run_start, len(ks)])
    rank = np.arange(len(ks)) - np.repeat(run_start, run_len)
    pos = (base_bt[bkt, tl][order] * 128 + rank) + \
        owner[order] * (NCHUNK * 128)

    LI = np.zeros(C * NCHUNK * 128, dtype=np.int16)
    DL = np.zeros(C * NCHUNK * 128, dtype=np.int16)
    CF = np.zeros(C * NCHUNK * 128, dtype=np.float32)
    LI[pos] = lidx[order]
    DL[pos] = dloc[order]
    CF[pos] = CO[order]
    # idx blocks: [C, NCALLS, 128, GCALL/16], wrapped + replicated in 8 bands
    IDX = LI.reshape(C, NCALLS, cfg.GCALL // 16, 16).transpose(0, 1, 3, 2)
    IDX = np.tile(IDX, (1, 1, 8, 1)).copy()
    DL = DL.reshape(C, NCHUNK, 128)
    CF = CF.reshape(C, NCHUNK, 128).astype(BF16)

    # pooling metadata
    gmins = []
    GL = np.full((C, SPAD), -1, dtype=np.int16)
    for c in range(C):
        bc = batch[c * NPC:(c + 1) * NPC]
        gmin = int(bc[0])
        span = int(bc[-1]) - gmin + 1
        assert span <= cfg.W, f"graph span {span} exceeds pool window {cfg.W}"
        gmins.append(gmin)
        GL[c, :NPC] = (bc - gmin).astype(np.int16)

    cnt = np.bincount(batch, minlength=cfg.G).astype(np.float32)
    cntinv = np.ones((1, cfg.GPAD), dtype=np.float32)
    cntinv[0, :cfg.G] = 1.0 / np.maximum(cnt, 1.0)

    # node features: padded full table (replicated, 128-wide) + transposed shard
    x_full = np.zeros((C * SPAD, 128), dtype=BF16)
    for c in range(C):
        x_full[c * SPAD:c * SPAD + NPC, :cfg.IN] = \
            x[c * NPC:(c + 1) * NPC].astype(BF16)

    eF = np.asarray(eFeature, dtype=np.float32)
    eFT = np.zeros((cfg.EX, cfg.GPAD), dtype=np.float32)
    eFT[:, :cfg.G] = eF.T

    w = {k: np.asarray(v, dtype=np.float32) for k, v in weights.items()}
    bfw = lambda a: np.ascontiguousarray(a.astype(BF16))
    col = lambda a: np.ascontiguousarray(a.reshape(-1, 1).astype(np.float32))
    shared = {
        "eFT": eFT, "cntinv": cntinv, "x_full": x_full,
        "W1": bfw(w["W1"]), "W2": bfw(w["W2"]), "W3": bfw(w["W3"]),
        "s1_Wp": bfw(w["s1_Wp"]),
        "s1_Wi": bfw(w["s1_Wi"]), "s1_Wo": bfw(w["s1_Wo"]),
        "s2_Wi": bfw(w["s2_Wi"]), "s2_Wo": bfw(w["s2_Wo"]),
        "s3_Wi": bfw(w["s3_Wi"]), "s3_Wo": bfw(w["s3_Wo"]),
        "bio1": col(w["s1_bi"] + w["s1_bo"]),
        "bio2": col(w["s2_bi"] + w["s2_bo"]),
        "bio3": col(w["s3_bi"] + w["s3_bo"]),
        "g1": col(w["g1"]), "g2": col(w["g2"]), "g3": col(w["g3"]),
        "be1": col(w["be1"]), "be2": col(w["be2"]), "be3": col(w["be3"]),
        "fc1": np.ascontiguousarray(w["fc1_W"]),
        "fc1_b": col(w["fc1_b"]),
        "fc3": np.ascontiguousarray(w["fc3_W"]),
        "fc3_b": col(w["fc3_b"]),
    }
    in_maps = []
    for c in range(C):
        m = dict(shared)
        m["idxblk"] = np.ascontiguousarray(IDX[c])      # [NCALLS,128,GCALL/16]
        m["dstloc"] = np.ascontiguousarray(DL[c].T)
        m["coef"] = np.ascontiguousarray(CF[c].T)
        m["glocal"] = np.ascontiguousarray(GL[c].reshape(T, 128).T)  # [128, T]
        m["xT"] = np.ascontiguousarray(
            x_full[c * SPAD:(c + 1) * SPAD, :cfg.IN].T)  # [IN, SPAD] bf16
        in_maps.append(m)
    params = {"K_bt": K_bt.tolist(), "NCHUNK": NCHUNK, "NCALLS": NCALLS,
              "NB": NB, "BROWS": BROWS,
              "call_bucket": call_bucket.tolist(), "gmins": gmins}
    return in_maps, params


# ---------------------------------------------------------------- builder
def build(cfg, params):
    C, T = cfg.C, cfg.T
    NCHUNK, NCALLS = params["NCHUNK"], params["NCALLS"]
    NB, BROWS = params["NB"], params["BROWS"]
    K_bt, call_bucket = params["K_bt"], params["call_bucket"]
    SPAD, IN, HID, EX, OUT = cfg.SPAD, cfg.IN, cfg.HID, cfg.EX, cfg.OUT
    W, GPAD, GX = cfg.W, cfg.GPAD, cfg.GX
    GCALL, GCH = cfg.GCALL, cfg.GCALL // 128
    gmins = params["gmins"]
    RG = [list(range(C))]
    CW = 512                                   # node-chunk width
    NCH = -(-SPAD // CW)                       # node chunks

    nc = bacc.Bacc("TRN2", target_bir_lowering=False, debug=False,
                   num_devices=C)
    SH = "Shared" if C > 4 else "Local"

    # ---- dram tensors
    din = {}
    def ein(name, shape, dt):
        din[name] = nc.dram_tensor(name, list(shape), dt, kind="ExternalInput").ap()
        return din[name]

    ein("idxblk", (NCALLS, 128, GCALL // 16), I16)
    ein("dstloc", (128, NCHUNK), I16)
    ein("coef", (128, NCHUNK), BF)
    ein("glocal", (128, T), I16)
    ein("x_full", (C * SPAD, 128), BF)
    ein("xT", (IN, SPAD), BF)
    ein("eFT", (EX, GPAD), F32)
    ein("cntinv", (1, GPAD), F32)
    for n in ("W1", "s1_Wp"):
        ein(n, (IN, HID), BF)
    for n in ("W2", "W3", "s1_Wi", "s1_Wo", "s2_Wi", "s2_Wo", "s3_Wi", "s3_Wo"):
        ein(n, (HID, HID), BF)
    for n in ("bio1", "bio2", "bio3", "g1", "g2", "g3", "be1", "be2", "be3",
              "fc1_b"):
        ein(n, (HID, 1), F32)
    ein("fc1", (HID * 3 + EX, HID), F32)
    ein("fc3", (HID, OUT), F32)
    ein("fc3_b", (OUT, 1), F32)

    out_d = nc.dram_tensor("out", [OUT, GPAD], F32, kind="ExternalOutput").ap()

    tables = [din["x_full"], None, None]
    tl_in = [None] * 3
    for l in (1, 2):
        tables[l] = nc.dram_tensor(f"table{l}", [C * SPAD, HID], BF,
                                   kind="Internal", addr_space=SH).ap()
    for l in (0, 1):
        tl_in[l] = nc.dram_tensor(f"tlin{l}", [SPAD, HID], BF,
                                  kind="Internal").ap()
    bn_i = [nc.dram_tensor(f"bni{l}", [128, 2], F32, kind="Internal").ap()
            for l in range(3)]
    bn_o = [nc.dram_tensor(f"bno{l}", [128, 2], F32, kind="Internal",
                           addr_space=SH).ap() for l in range(3)]
    pool_i = nc.dram_tensor("pooli", [128, 3 * W], F32, kind="Internal").ap()
    pool_o = nc.dram_tensor("poolo", [C, 128, 3 * W], F32, kind="Internal",
                            addr_space=SH).ap()

    from contextlib import ExitStack
    with tile.TileContext(nc) as tc, ExitStack() as stack:
        PSa = stack.enter_context(tc.tile_pool(name="psa", bufs=2, space="PSUM"))
        PSb = stack.enter_context(tc.tile_pool(name="psb", bufs=2, space="PSUM"))
        PSt = stack.enter_context(tc.tile_pool(name="pst", bufs=2, space="PSUM"))
        PSL = stack.enter_context(tc.tile_pool(name="psuml", bufs=2, space="PSUM"))
        PP = stack.enter_context(tc.tile_pool(name="persist", bufs=1))

        def load(name, shape, dt, tag=None):
            t = PP.tile(list(shape), dt, tag=tag or name)
            nc.sync.dma_start(out=t[:], in_=din[name][:])
            return t

        dstloc = load("dstloc", (128, NCHUNK), I16)
        coef = load("coef", (128, NCHUNK), BF)
        glocal = load("glocal", (128, T), I16)
        eFT = load("eFT", (EX, GPAD), F32)
        cntinv = load("cntinv", (1, GPAD), F32)
        Wl_t = [load("W1", (IN, HID), BF), load("W2", (HID, HID), BF),
                load("W3", (HID, HID), BF)]
        s1Wp = load("s1_Wp", (IN, HID), BF)
        Wi_t = [load("s1_Wi", (HID, HID), BF), load("s2_Wi", (HID, HID), BF),
                load("s3_Wi", (HID, HID), BF)]
        Wo_t = [load("s1_Wo", (HID, HID), BF), load("s2_Wo", (HID, HID), BF),
                load("s3_Wo", (HID, HID), BF)]
        bio_t = [load(f"bio{l+1}", (HID, 1), F32) for l in range(3)]
        g_t = [load(f"g{l+1}", (HID, 1), F32) for l in range(3)]
        be_t = [load(f"be{l+1}", (HID, 1), F32) for l in range(3)]
        fc1_p = []
        for lb in range(3):
            t_ = PP.tile([HID, HID], F32, tag=f"fc1_{lb}")
            nc.sync.dma_start(out=t_[:],
                              in_=din["fc1"][lb * HID:(lb + 1) * HID, :])
            fc1_p.append(t_)
        fc1_e = PP.tile([EX, HID], F32, tag="fc1_e")
        nc.sync.dma_start(out=fc1_e[:], in_=din["fc1"][3 * HID:, :])
        fc1_b = load("fc1_b", (HID, 1), F32)
        fc3 = load("fc3", (HID, OUT), F32)
        fc3_b = load("fc3_b", (OUT, 1), F32)

        ident = PP.tile([128, 128], BF, tag="ident")
        make_identity(nc, ident[:])
        iota128 = PP.tile([128, 128], I16, tag="iota128")
        nc.gpsimd.iota(iota128[:], pattern=[[1, 128]], base=0,
                       channel_multiplier=0)
        iotaW = PP.tile([128, W], I16, tag="iotaW")
        nc.gpsimd.iota(iotaW[:], pattern=[[1, W]], base=0, channel_multiplier=0)
        ones1 = PP.tile([1, 128], F32, tag="ones1")
        nc.vector.memset(ones1[:], 1.0)
        epsc = PP.tile([128, 1], F32, tag="epsc")
        nc.vector.memset(epsc[:], EPS)

        y_sb = PP.tile([HID, SPAD], BF, tag="y")
        hin = PP.tile([HID, SPAD], BF, tag="hin")
        hn = PP.tile([HID, SPAD], BF, tag="hn")
        big = PP.tile([128, SPAD], BF, tag="big")      # agg / hT shared scratch
        pooled = PP.tile([128, 3 * W], F32, tag="pooled")
        ysums = PP.tile([128, T], F32, tag="ysums")
        y2sums = PP.tile([128, NCH], F32, tag="y2sums")
        bnloc = PP.tile([128, 2], F32, tag="bnloc")
        bnst = PP.tile([128, 2], F32, tag="bnst")
        stat = PP.tile([128, 8], F32, tag="stat")      # mu|ex2|musq|var|sd|rs|A|B

        AL = mybir.AluOpType
        AF = mybir.ActivationFunctionType

        # ---- initial skip input: hin = (x @ s1_Wp)^T
        with tc.tile_pool(name="xtp", bufs=1) as XP:
            xT = XP.tile([IN, SPAD], BF, tag="xT")
            nc.sync.dma_start(out=xT[:], in_=din["xT"][:])
            for cix in range(NCH):
                cs = slice(cix * CW, min((cix + 1) * CW, SPAD))
                ps = PSb.tile([HID, CW], F32, tag="ps512")
                nc.tensor.matmul(out=ps[:, :cs.stop - cs.start], lhsT=s1Wp[:],
                                 rhs=xT[:, cs], start=True, stop=True)
                nc.vector.tensor_copy(out=hin[:, cs],
                                      in_=ps[:, :cs.stop - cs.start])

        lstack = ExitStack()
        GV = lstack.enter_context(tc.tile_pool(name="gather", bufs=2))
        GS = lstack.enter_context(tc.tile_pool(name="sbuild", bufs=2))
        WK = lstack.enter_context(tc.tile_pool(name="work", bufs=2))
        for l in range(3):
            F_in = IN if l == 0 else HID
            table = tables[l]

            # ---- A: bucketed gather + segmented sum -> big[:F_in, :] (bf16)
            Vt = [None]
            St = [None]

            def open_call(k):
                ixt = GS.tile([128, GCALL // 16], I16, tag="ixt")
                nc.sync.dma_start(out=ixt[:], in_=din["idxblk"][k, :, :])
                V = GV.tile([128, GCALL], BF, tag="V")
                b_ = call_bucket[k]
                nc.gpsimd.dma_gather(
                    out_ap=V[:].rearrange("p (k f) -> p k f", f=128),
                    in_ap=table[b_ * BROWS:(b_ + 1) * BROWS, :],
                    idxs_ap=ixt[:], num_idxs=GCALL, num_idxs_reg=GCALL,
                    elem_size=128, single_packet=False)
                S = GS.tile([128, GCALL], BF, tag="S")
                S3 = S[:].rearrange("p (k f) -> p k f", f=128)
                nc.vector.tensor_tensor(
                    out=S3, in0=iota128[:].unsqueeze(1)
                        .broadcast_to([128, GCH, 128]),
                    in1=dstloc[:, k * GCH:(k + 1) * GCH].unsqueeze(-1)
                        .broadcast_to([128, GCH, 128]),
                    op=AL.is_equal)
                nc.vector.tensor_tensor(
                    out=S3, in0=S3,
                    in1=coef[:, k * GCH:(k + 1) * GCH].unsqueeze(-1)
                        .broadcast_to([128, GCH, 128]),
                    op=AL.mult)
                Vt[0], St[0] = V, S

            evicted = [False] * T
            g = 0
            cur_call = -1
            for b_ in range(NB):
                for t in range(T):
                    kbt = K_bt[b_][t]
                    if kbt == 0:
                        continue
                    pa = PSa.tile([F_in, 128], F32, tag="psA")
                    for j in range(kbt):
                        k = g // GCH
                        if k != cur_call:
                            open_call(k)
                            cur_call = k
                        blk = g % GCH
                        nc.tensor.matmul(
                            out=pa[:],
                            lhsT=Vt[0][:, blk * 128:blk * 128 + F_in],
                            rhs=St[0][:, blk * 128:(blk + 1) * 128],
                            start=(j == 0), stop=(j == kbt - 1))
                        g += 1
                    dstv = big[:F_in, t * 128:(t + 1) * 128]
                    if not evicted[t]:
                        nc.vector.tensor_copy(out=dstv, in_=pa[:])
                        evicted[t] = True
                    else:
                        nc.vector.tensor_tensor(out=dstv, in0=dstv, in1=pa[:],
                                                op=AL.add)
            for t in range(T):
                if not evicted[t]:
                    nc.vector.memset(big[:F_in, t * 128:(t + 1) * 128], 0.0)

            # ---- B: y = W_l.T @ agg  (+ per-tile sum accum)
            for cix in range(NCH):
                cs = slice(cix * CW, min((cix + 1) * CW, SPAD))
                w_ = cs.stop - cs.start
                ps = PSb.tile([HID, CW], F32, tag="ps512")
                nc.tensor.matmul(out=ps[:, :w_], lhsT=Wl_t[l][:],
                                 rhs=big[:F_in, cs], start=True, stop=True)
                nc.scalar.activation(out=y_sb[:, cs], in_=ps[:, :w_],
                                     func=AF.Identity,
                                     accum_out=ysums[:, cix:cix + 1])
            # ---- C: sum of squares
            for cix in range(NCH):
                cs = slice(cix * CW, min((cix + 1) * CW, SPAD))
                sq = WK.tile([HID, CW], BF, tag="sq")
                nc.scalar.activation(out=sq[:, :cs.stop - cs.start],
                                     in_=y_sb[:, cs], func=AF.Square,
                                     accum_out=y2sums[:, cix:cix + 1])
            # ---- D: BN stats allreduce + A,B factors
            nc.vector.tensor_reduce(out=bnloc[:, 0:1], in_=ysums[:, :NCH],
                                    axis=mybir.AxisListType.X, op=AL.add)
            nc.vector.tensor_reduce(out=bnloc[:, 1:2], in_=y2sums[:, :NCH],
                                    axis=mybir.AxisListType.X, op=AL.add)
            nc.gpsimd.dma_start(out=bn_i[l][:], in_=bnloc[:])
            nc.gpsimd.collective_compute(
                "AllReduce", AL.add, replica_groups=RG,
                ins=[bn_i[l][:]], outs=[bn_o[l][:]])
            nc.sync.dma_start(out=bnst[:], in_=bn_o[l][:])
            mu, ex2, musq, var = (stat[:, i:i + 1] for i in range(4))
            sd, rs, Af, Bf = (stat[:, i:i + 1] for i in range(4, 8))
            inv_n = 1.0 / float(cfg.N)
            nc.vector.tensor_scalar_mul(mu, bnst[:, 0:1], inv_n)
            nc.vector.tensor_scalar_mul(ex2, bnst[:, 1:2], inv_n)
            nc.vector.tensor_tensor(out=musq, in0=mu, in1=mu, op=AL.mult)
            nc.vector.tensor_tensor(out=var, in0=ex2, in1=musq, op=AL.subtract)
            nc.scalar.activation(out=sd, in_=var, func=AF.Sqrt, bias=epsc[:])
            nc.vector.reciprocal(rs, sd)
            nc.vector.tensor_tensor(out=Af, in0=rs, in1=g_t[l][:], op=AL.mult)
            tmp = stat[:, 2:3]  # reuse musq slot
            nc.vector.tensor_tensor(out=tmp, in0=mu, in1=Af, op=AL.mult)
            nc.vector.tensor_tensor(out=Bf, in0=be_t[l][:], in1=tmp,
                                    op=AL.subtract)

            # ---- E: relu-affine, gates, blend -> hn
            for cix in range(NCH):
                cs = slice(cix * CW, min((cix + 1) * CW, SPAD))
                w_ = cs.stop - cs.start
                ho = WK.tile([HID, CW], BF, tag="ho")
                nc.scalar.activation(out=ho[:, :w_], in_=y_sb[:, cs],
                                     func=AF.Relu, bias=Bf, scale=Af)
                ps = PSb.tile([HID, CW], F32, tag="ps512")
                nc.tensor.matmul(out=ps[:, :w_], lhsT=Wo_t[l][:],
                                 rhs=ho[:, :w_], start=True, stop=False)
                nc.tensor.matmul(out=ps[:, :w_], lhsT=Wi_t[l][:],
                                 rhs=hin[:, cs], start=False, stop=True)
                z = WK.tile([HID, CW], BF, tag="z")
                nc.scalar.activation(out=z[:, :w_], in_=ps[:, :w_],
                                     func=AF.Sigmoid, bias=bio_t[l][:])
                d = WK.tile([HID, CW], BF, tag="sq")
                nc.vector.tensor_tensor(out=d[:, :w_], in0=ho[:, :w_],
                                        in1=hin[:, cs], op=AL.subtract)
                nc.vector.tensor_tensor(out=d[:, :w_], in0=z[:, :w_],
                                        in1=d[:, :w_], op=AL.mult)
                nc.vector.tensor_tensor(out=hn[:, cs], in0=d[:, :w_],
                                        in1=hin[:, cs], op=AL.add)

            # ---- F: transpose tiles -> big (hT), pooling
            big3 = big[:].rearrange("p (t f) -> p t f", f=128)
            pps = PSL.tile([HID, W], F32, tag="poolps")
            for t in range(T):
                pst = PSt.tile([128, 128], BF, tag="psT")
                nc.tensor.transpose(out=pst[:], in_=hn[:, t * 128:(t + 1) * 128],
                                    identity=ident[:])
                nc.vector.tensor_copy(out=big3[:, t, :], in_=pst[:])
                P = WK.tile([128, W], BF, tag="P")
                nc.vector.tensor_tensor(
                    out=P[:], in0=iotaW[:],
                    in1=glocal[:, t:t + 1].broadcast_to([128, W]),
                    op=AL.is_equal)
                nc.tensor.matmul(out=pps[:], lhsT=big3[:, t, :], rhs=P[:],
                                 start=(t == 0), stop=(t == T - 1))
            nc.vector.tensor_copy(out=pooled[:HID, l * W:(l + 1) * W],
                                  in_=pps[:])

            # ---- G: write padded table shard + allgather next table
            if l < 2:
                nc.sync.dma_start(
                    out=tl_in[l][:].rearrange("(t n) f -> n t f", n=128),
                    in_=big3[:, :, :])
                nc.gpsimd.collective_compute(
                    "AllGather", AL.bypass, replica_groups=RG,
                    ins=[tl_in[l][:]], outs=[tables[l + 1][:]])

            hin, hn = hn, hin   # next layer's skip input is this layer's output

        # ---- head
        lstack.close()
        HD = stack.enter_context(tc.tile_pool(name="head", bufs=1))
        HW = stack.enter_context(tc.tile_pool(name="hwork", bufs=2))
        pglob = HD.tile([128, 3 * GX], F32, tag="pglob")
        cb = HD.tile([128, GPAD], F32, tag="cb")
        h1g = HD.tile([HID, GPAD], F32, tag="h1g")
        out_sb = HD.tile([OUT, GPAD], F32, tag="out_sb")
        nc.sync.dma_start(out=pool_i[:], in_=pooled[:])
        nc.gpsimd.collective_compute("AllGather", AL.bypass, replica_groups=RG,
                                     ins=[pool_i[:]], outs=[pool_o[:]])
        nc.vector.memset(pglob[:], 0.0)
        pg3 = pglob[:].rearrange("p (l g) -> p l g", g=GX)
        for c in range(C):
            blk = HW.tile([128, 3 * W], F32, tag="blk")
            nc.sync.dma_start(out=blk[:], in_=pool_o[c, :, :])
            for l in range(3):
                dstv = pg3[:, l, gmins[c]:gmins[c] + W]
                nc.vector.tensor_tensor(out=dstv, in0=dstv,
                                        in1=blk[:, l * W:(l + 1) * W],
                                        op=AL.add)
        # broadcast cnt_inv across partitions via ones outer product
        for cc in range(-(-GPAD // CW)):
            gs_ = slice(cc * CW, min((cc + 1) * CW, GPAD))
            w_ = gs_.stop - gs_.start
            pcb = PSb.tile([128, CW], F32, tag="ps512")
            nc.tensor.matmul(out=pcb[:, :w_], lhsT=ones1[:],
                             rhs=cntinv[:, gs_], start=True, stop=True)
            nc.vector.tensor_copy(out=cb[:, gs_], in_=pcb[:, :w_])
        for l in range(3):
            for cc in range(-(-GPAD // CW)):
                gs_ = slice(cc * CW, min((cc + 1) * CW, GPAD))
                nc.vector.tensor_tensor(out=pg3[:, l, gs_], in0=pg3[:, l, gs_],
                                        in1=cb[:, gs_], op=AL.mult)
        for cc in range(-(-GPAD // CW)):
            gs_ = slice(cc * CW, min((cc + 1) * CW, GPAD))
            w_ = gs_.stop - gs_.start
            ps1 = PSb.tile([HID, CW], F32, tag="ps512")
            for l in range(3):
                nc.tensor.matmul(out=ps1[:, :w_], lhsT=fc1_p[l][:],
                                 rhs=pg3[:, l, gs_], start=(l == 0), stop=False)
            nc.tensor.matmul(out=ps1[:, :w_], lhsT=fc1_e[:],
                             rhs=eFT[:, gs_], start=False, stop=True)
            nc.scalar.activation(out=h1g[:, gs_], in_=ps1[:, :w_],
                                 func=AF.Relu, bias=fc1_b[:])
            ps2 = PSb.tile([OUT, CW], F32, tag="ps512")
            nc.tensor.matmul(out=ps2[:, :w_], lhsT=fc3[:], rhs=h1g[:, gs_],
                             start=True, stop=True)
            nc.scalar.activation(out=out_sb[:, gs_], in_=ps2[:, :w_],
                                 func=AF.Identity, bias=fc3_b[:])
        nc.sync.dma_start(out=out_d[:], in_=out_sb[:])

    nc.compile()
    return nc


# ---------------------------------------------------------------- entry
_WEIGHT_NAMES = [
    "W1", "b1", "W2", "b2", "W3", "b3", "g1", "be1", "g2", "be2", "g3", "be3",
    "s1_Wp", "s1_Wi", "s1_bi", "s1_Wo", "s1_bo", "s2_Wi", "s2_bi", "s2_Wo",
    "s2_bo", "s3_Wi", "s3_bi", "s3_Wo", "s3_bo", "fc1_W", "fc1_b", "fc3_W",
    "fc3_b",
]


def run(cfg, x, eFeature, edge_index, batch, weights, sim=False,
        want_results=True):
    in_maps, params = preprocess(cfg, x, eFeature, edge_index, batch, weights)
    nc = build(cfg, params)
    if sim:
        from concourse.bass_interp import MultiCoreSim
        ms = MultiCoreSim(nc, num_cores=cfg.C, trace=False)
        cores = list(ms.cores.values())
        for c, cs in enumerate(cores):
            for k, v in in_maps[c].items():
                cs.tensor(k)[:] = v
        ms.simulate(check_with_hw=False)
        out = np.asarray(cores[0].tensor("out"))
    else:
        res = bass_utils.run_bass_kernel_spmd(
            nc, in_maps, core_ids=list(range(cfg.C)))
        out = res.results[0]["out"]
    return np.ascontiguousarray(out[:, :cfg.G].T.astype(np.float32)), nc, in_maps


def kernel(**inputs):
    cfg = FULL
    weights = {k: v for k, v in inputs.items()
               if k not in ("x", "eFeature", "edge_index", "batch")}
    out, _, _ = run(cfg, inputs["x"], inputs["eFeature"],
                    inputs["edge_index"], inputs["batch"], weights)
    return out
